# revision 4
# baseline (speedup 1.0000x reference)
"""EnhancedEntityNBFNet (B=2, K=33, N=50000, E=800000, R=200, D=64, L=3)
on 8 Trainium2 NeuronCores — latency-optimized frontier-compacted Bass kernel.

Same math as the baseline (see reference): zero-bias invariance keeps node
state supported on the tiny h0 out-frontier, so each core computes dense
linear algebra over compacted active sets.  This version is restructured for
serial LATENCY (the workload is ~40 dependent small ops):

  - relation table compacted to the <=126 types actually used by this core's
    edges -> every rel-contraction is a single <=128-partition matmul
  - one bf16 input blob (halves DMA bytes; matmuls run at 1 cycle/row
    instead of fp32's 4); an embedded fp32-bitcast region carries eps/qvT
  - qv / qvT / eps / ones / h0-indicators are host-gathered blob constants
    (indexing only -- no host float math)
  - [G_src | G_prev] fused per layer: one matmul yields gx and the residual
    gather stacked in PSUM; 5 hand-carved PSUM banks, no pool-reuse sems
  - the MLP transpose uses the DVE 32x32 stream transpose (no PE transpose,
    no identity); the h matmul contracts per 32-block
  - framework const-ap memsets are skipped (nothing reads them), pulling the
    program start barrier from ~660ns to ~300ns; a dummy Sqrt hoists the
    1283ns activation-table load ahead of the input DMA
  - a prepared dma_scatter_add output path exists behind KN_TRIG_OUT
    (~1.1us faster tail; passed one HW run at 256B elements with a
    device-side zero-fill and a drain-gated trigger, but showed an
    intermittent runtime failure on a repeat run, so the deterministic
    plain output DMA stays the default)
  - hardware pitfalls found on the way: all matmuls of one PSUM accumulation
    group must share a tile position (base partitions), and one DVE op may
    read at most one PSUM operand

Sharding: core c handles batch b = c//4 and a quarter of the K tail
candidates; host concatenates per-core score rows.  Frontiers that don't fit
(any active dim > 128, > 126 used types, or KC > 32) fall back to the
block-tiled general program; a numpy path guards the zero-bias invariance.
"""
import sys

import numpy as np

for _p in ("/opt/trn_rl_repo", "/root/.axon_site/_ro/trn_rl_repo"):
    if _p not in sys.path:
        sys.path.insert(0, _p)

from contextlib import ExitStack

import concourse.bacc as bacc
import concourse.tile as tile
from concourse import mybir
from concourse.bass_utils import run_bass_kernel_spmd
from concourse.masks import make_identity

F32 = mybir.dt.float32
BF16 = mybir.dt.bfloat16
I16 = mybir.dt.int16
P = 128
D = 64          # hidden dim
RP = 256        # full relation table rows, padded (R=200 -> 256), general path
L = 3           # layers
EPS = 1e-5
N_CORES = 8
SCW = 64        # score DRAM width (scatter elem = 256B)


def _pad16(n: int) -> int:
    return max(16, ((int(n) + 15) // 16) * 16)


def _blk(n):
    return [(o, min(P, n - o)) for o in range(0, n, P)]


# --------------------------------------------------------------------------
# host-side integer prep: frontier sets + per-core used relation types
# --------------------------------------------------------------------------

def _prep_host(rel, batch, edge_index, edge_type):
    src = np.asarray(edge_index[0], np.int64)
    dst = np.asarray(edge_index[1], np.int64)
    et = np.asarray(edge_type, np.int64)
    B = rel.shape[0]
    K = batch.shape[1]

    per_batch = []
    for b in range(B):
        h0 = int(batch[b, 0, 0])
        r0 = int(batch[b, 0, 2])
        e1 = np.nonzero(src == h0)[0]
        V1 = np.unique(np.concatenate([[h0], dst[e1]]))
        A2 = np.union1d(V1, dst[np.isin(src, V1)])
        per_batch.append(dict(h0=h0, r0=r0, e1=e1, V1=V1, A2=A2))

    cpb = N_CORES // B  # cores per batch
    chunks = np.array_split(np.arange(K), cpb)
    cores = []
    for c in range(N_CORES):
        b = c // cpb
        pb = per_batch[b]
        chunk = chunks[c % cpb]
        Tc = batch[b, chunk, 1].astype(np.int64)
        e3 = np.nonzero(np.isin(dst, Tc) & np.isin(src, pb["A2"]))[0]
        V2 = np.unique(np.concatenate([Tc, src[e3]]))
        e2 = np.nonzero(np.isin(dst, V2) & np.isin(src, pb["V1"]))[0]
        types = np.unique(np.concatenate(
            [et[pb["e1"]], et[e2], et[e3], [pb["r0"]]]))
        cores.append(dict(b=b, Tc=Tc, e2=e2, e3=e3, V2=V2, chunk=chunk,
                          types=types))

    dims = dict(
        M1=_pad16(max(len(pb["V1"]) for pb in per_batch)),
        Q1=_pad16(max(len(pb["e1"]) for pb in per_batch)),
        M2=_pad16(max(len(ci["V2"]) for ci in cores)),
        Q2=_pad16(max(len(ci["e2"]) for ci in cores)),
        Q3=_pad16(max(len(ci["e3"]) for ci in cores)),
        KC=_pad16(max(len(ci["Tc"]) for ci in cores)),
        RU=_pad16(max(len(ci["types"]) for ci in cores) + 2),
    )
    return per_batch, cores, dims, (src, dst, et)


def _flags(inputs):
    return dict(
        ln_affine=not (np.all(np.asarray(inputs["ln_g"]) == 1)
                       and np.all(np.asarray(inputs["ln_b"]) == 0)),
        layer_bias=bool(np.any(np.asarray(inputs["layer_b"]) != 0)),
        mlp_bias=bool(np.any(np.asarray(inputs["mlp_b1"]) != 0)
                      or np.any(np.asarray(inputs["mlp_b2"]) != 0)),
    )


# --------------------------------------------------------------------------
# fast path: per-core packed blobs (2 input DMAs, parallel queues)
# --------------------------------------------------------------------------

def _seg(cols):
    out, off = {}, 0
    for name, w in cols:
        out[name] = (off, w)
        off += w
    return out, off


def _blob_layout(dims, flags):
    M1, M2, Q2, Q3, KC = (dims[k] for k in ("M1", "M2", "Q2", "Q3", "KC"))
    # One bf16 blob.  matmul lhsT/rhs must share a base partition in
    # {0, 32, 64}, and every matmul of one PSUM accumulation group must use
    # the same tile position -- all row-vector constants live at p0.  gg2/gg3 hold
    # [G_src | G_prev] so one matmul yields gx and xp stacked; their G_prev
    # column range doubles as the cat-bottom rhs.  The trailing f32 segment
    # is a bitcast region (eps column + qvT column), 4-byte aligned.
    XO2 = ((Q2 + 31) // 32) * 32   # xp rows land at a legal partition base
    XO3 = ((Q3 + 31) // 32) * 32
    cols = [("rel", D), ("c1", M1), ("w0", D), ("msc", D), ("h01", M1),
            ("tm2", Q2), ("tm3", Q3), ("gg2", XO2 + M2), ("s2", M2),
            ("gg3", XO3 + KC), ("s3", KC),
            ("w1", D), ("w2", D), ("mw1h", 2 * D), ("mw1b", D),
            ("h02", M2), ("h03", KC), ("ones", KC)]
    if flags["layer_bias"]:
        cols += [(f"lb{l}", D) for l in range(3)]
    if flags["mlp_bias"]:
        cols.append(("mb1", D))
    if flags["ln_affine"]:
        cols += [(f"lga{l}", D) for l in range(3)]
        cols += [(f"lnba{l}", D) for l in range(3)]
    cols += [("qvT32", 2), ("eps32", 2), ("qvTb", 1), ("mw2", 1)]
    seg, C = _seg(cols)
    return seg, C


def _prep_blobs(inputs, rel, per_batch, cores, dims, graph, flags):
    import ml_dtypes
    src, dst, et = graph
    M1, M2, Q2, Q3, KC, RU = (dims[k] for k in
                              ("M1", "M2", "Q2", "Q3", "KC", "RU"))
    seg, Cc = _blob_layout(dims, flags)
    lw = np.asarray(inputs["layer_w"], np.float32)
    lbv = np.asarray(inputs["layer_b"], np.float32)
    lng = np.asarray(inputs["ln_g"], np.float32)
    lnb = np.asarray(inputs["ln_b"], np.float32)
    mw1 = np.asarray(inputs["mlp_w1"], np.float32)
    mb1 = np.asarray(inputs["mlp_b1"], np.float32)
    mw2 = np.asarray(inputs["mlp_w2"], np.float32)

    in_maps = []
    for ci in cores:
        pb = per_batch[ci["b"]]
        h0, r0, e1, V1 = pb["h0"], pb["r0"], pb["e1"], pb["V1"]
        Tc, e2, e3, V2 = ci["Tc"], ci["e2"], ci["e3"], ci["V2"]
        types = ci["types"]
        b = ci["b"]
        pos1 = {n: i for i, n in enumerate(V1)}
        pos2 = {n: i for i, n in enumerate(V2)}
        tmap = {t: i for i, t in enumerate(types)}
        nu = len(types)
        iONE = RU - 1
        q1, q2, q3, kc = len(e1), len(e2), len(e3), len(Tc)

        qv = np.asarray(rel[b, r0], np.float32)           # [D]
        h01 = np.zeros(M1, np.float32)
        h01[pos1[h0]] = 1.0
        h02 = np.zeros(M2, np.float32)
        if h0 in pos2:
            h02[pos2[h0]] = 1.0
        h03 = np.pad((Tc == h0).astype(np.float32), (0, KC - kc))

        B1 = np.zeros((P, Cc), ml_dtypes.bfloat16)

        def put(name, arr, r0_=0):
            o, w = seg[name]
            a = np.asarray(arr, np.float32)
            B1[r0_: r0_ + a.shape[0], o: o + a.shape[1]] = a.astype(
                ml_dtypes.bfloat16)

        def put32(name, arr):
            o, w = seg[name]
            a = np.asarray(arr, np.float32)
            B1[:, o: o + w].view(np.float32)[: a.shape[0], : a.shape[1]] = a

        relc = np.zeros((RU, D), np.float32)
        relc[:nu] = rel[b, types]
        relc[iONE] = 1.0
        put("rel", relc)
        c1a = np.zeros((RU, M1), np.float32)
        if q1:
            np.add.at(c1a, ([tmap[t] for t in et[e1]],
                            [pos1[n] for n in dst[e1]]), 1.0)
        c1a[iONE] = h01
        put("c1", c1a)
        put("w0", lw[0])
        put("msc", qv.reshape(1, -1))
        put("h01", h01.reshape(1, -1))

        XO2 = ((Q2 + 31) // 32) * 32
        XO3 = ((Q3 + 31) // 32) * 32
        tm2 = np.zeros((RU, Q2), np.float32)
        gg2 = np.zeros((M1, XO2 + M2), np.float32)
        s2 = np.zeros((Q2, M2), np.float32)
        if q2:
            tm2[[tmap[t] for t in et[e2]], np.arange(q2)] = 1.0
            gg2[[pos1[n] for n in src[e2]], np.arange(q2)] = 1.0
            s2[np.arange(q2), [pos2[n] for n in dst[e2]]] = 1.0
        for n in V2:
            if n in pos1:
                gg2[pos1[n], XO2 + pos2[n]] = 1.0
        tm3 = np.zeros((RU, Q3), np.float32)
        gg3 = np.zeros((M2, XO3 + KC), np.float32)
        s3 = np.zeros((Q3, KC), np.float32)
        if q3:
            tm3[[tmap[t] for t in et[e3]], np.arange(q3)] = 1.0
            gg3[[pos2[n] for n in src[e3]], np.arange(q3)] = 1.0
            s3[:q3, :kc] = (dst[e3][:, None] == Tc[None, :]).astype(np.float32)
        gg3[[pos2[n] for n in Tc], XO3 + np.arange(kc)] = 1.0

        put("tm2", tm2)
        put("tm3", tm3)
        put("gg2", gg2)
        put("s2", s2)
        put("gg3", gg3)
        put("s3", s3)
        put("w1", lw[1])
        put("w2", lw[2])
        put("mw1h", np.concatenate([mw1[0:32], mw1[32:64]], axis=1))
        put("mw1b", mw1[D:])
        put("mw2", mw2.reshape(-1, 1))
        put("h02", h02.reshape(1, -1))
        put("h03", h03.reshape(1, -1))
        put("ones", np.ones((1, KC), np.float32))
        if flags["layer_bias"]:
            for l in range(L):
                put(f"lb{l}", lbv[l].reshape(1, -1))
        if flags["mlp_bias"]:
            put("mb1", mb1.reshape(1, -1))
        if flags["ln_affine"]:
            for l in range(L):
                put(f"lga{l}", lng[l].reshape(1, -1))
                put(f"lnba{l}", lnb[l].reshape(1, -1))
        put32("qvT32", qv.reshape(-1, 1))
        put32("eps32", np.full((P, 1), EPS, np.float32))
        put("qvTb", qv.reshape(-1, 1))

        in_maps.append(dict(blob1=np.ascontiguousarray(B1)))
    return in_maps, (seg, Cc)


import contextlib


@contextlib.contextmanager
def _skip_constap_memsets():
    """Skip Bacc.__init__'s const-ap SBUF memsets (4 Pool-engine ops that
    delay the program start barrier).  Nothing in the fast-path program
    reads the const-ap tensors (all scalar operands are immediates or
    blob-backed APs)."""
    import concourse.bass as bassmod
    cls = bassmod.BassGpSimd

    def _noop_memset(self, ap, constant):
        return None

    cls.memset = _noop_memset
    try:
        yield
    finally:
        del cls.memset


def _build_nc_fast(dims, lay, flags):
    M1, M2, Q2, Q3, KC, RU = (dims[k] for k in
                              ("M1", "M2", "Q2", "Q3", "KC", "RU"))
    import os
    seg, Cc = lay
    mlp_b2 = flags.get("mlp_b2_val", 0.0)
    if os.environ.get("KN_NO_MEMSET_SKIP"):
        nc = bacc.Bacc()
    else:
        with _skip_constap_memsets():
            nc = bacc.Bacc()
    blob1 = nc.declare_dram_parameter("blob1", [P, Cc], BF16, isOutput=False)
    score = nc.declare_dram_parameter("score", [1, SCW], F32, isOutput=True)

    with ExitStack() as ctx:
        tc = ctx.enter_context(tile.TileContext(nc))
        const = ctx.enter_context(tc.tile_pool(name="const", bufs=1))
        tmp = ctx.enter_context(tc.tile_pool(name="tmp", bufs=2))
        pps = ctx.enter_context(tc.tile_pool(name="pps", bufs=1, space="PSUM"))
        # 5 full PSUM banks, manually carved into disjoint regions: no
        # buffer-reuse WAR semaphores anywhere.  cat2/cat3 keep a matmul
        # accumulation group OPEN from their early boundary-init to the late
        # msg^T S accumulate, and an open group owns its bank's zero region —
        # so each cat lives in its own bank.
        bankA = pps.tile([P, 512], F32, tag="bankA")   # cat1
        bankB = pps.tile([P, 512], F32, tag="bankB")   # u1|u2|u3|h|qwb|sc
        bankC = pps.tile([P, 512], F32, tag="bankC")   # x0|gxp2|gxp3|tr2|tr3
        bankD = pps.tile([P, 512], F32, tag="bankD")   # cat2
        bankE = pps.tile([P, 512], F32, tag="bankE")   # cat3

        # ---- single input DMA on the SP HWDGE queue
        t1 = const.tile([P, Cc], BF16, tag="t1")
        nc.sync.dma_start(out=t1[:, :], in_=blob1[0:P, :])

        def S(name, r0_=0, rows=P):
            o, w = seg[name]
            return t1[r0_: r0_ + rows, o: o + w]

        qv_row = S("msc", 0, 1)[:, :D]           # [1, D] bf16 at p0
        h01_row = S("h01", 0, 1)[:, :M1]
        h02_row = S("h02", 0, 1)[:, :M2]
        h03_row = S("h03", 0, 1)[:, :KC]
        ones_row = S("ones", 0, 1)[:, :KC]
        qvT = S("qvT32", 0, D).bitcast(F32)      # [D, 1] f32
        eps_t = S("eps32").bitcast(F32)          # [P, 1] f32

        # ---- output: prepared SWDGE scatter descriptor, triggered at the end
        use_trig = bool(os.environ.get("KN_TRIG_OUT"))
        scsrc = const.tile([P, SCW], F32, tag="scsrc")
        nc.vector.memset(scsrc[0:1, :], 0.0)
        if use_trig:
            # Prepared scatter-add, triggered after the score copy: descgen
            # and the DGE delay leave the critical path.  scatter ADDs, and
            # the PJRT path does not pre-zero output DRAM, so zero the score
            # row ourselves with an early DMA (done ~9us before the trigger).
            nc.sync.dma_start(out=score[0:1, :], in_=scsrc[0:1, :])
            idxs = const.tile([16, 16], I16, tag="idxs")
            nc.gpsimd.memset(idxs[:, :], -1)
            nc.gpsimd.memset(idxs[0:1, 0:1], 0)
            dma_sem = nc.alloc_semaphore("score_dma")
            csem = nc.alloc_semaphore("score_src")
            nc.gpsimd.dma_scatter_add(
                score[:, :], scsrc[:, :].rearrange("p (a b) -> p a b", a=1),
                idxs[:, 0:1], 1, 1, SCW, prepare_only=True, sem=dma_sem)

        # ---- Act table warm-up: a dummy Sqrt right at program start makes
        # the tc framework emit LoadActFuncSet (1283ns) before the input DMA
        # completes instead of right before layer-1's Sqrt.
        dm = const.tile([1, 1], F32, tag="dm")
        nc.vector.memset(dm[:, :], 0.0)
        nc.scalar.activation(out=dm[:1, :1], in_=dm[:1, :1],
                             func=mybir.ActivationFunctionType.Sqrt,
                             bias=dm[:1, 0:1], scale=1.0)

        # ---- DVE-side constants off the critical path
        x3 = const.tile([32, D], BF16, tag="x3")
        nc.vector.memset(x3[:, :], 0.0)
        w0q = const.tile([P, D], BF16, tag="w0q")
        nc.vector.tensor_copy(out=w0q[:, :], in_=S("w0"))
        nc.vector.tensor_scalar_mul(out=w0q[:D, :], in0=w0q[:D, :],
                                    scalar1=qvT)

        if flags["ln_affine"]:
            gbc, bbc = [], []
            ones1 = ones_row[:, :1].to_broadcast([1, P])
            for l in range(L):
                g_ps = bankA[:, 384: 384 + D]
                nc.tensor.matmul(out=g_ps, lhsT=ones1,
                                 rhs=S(f"lga{l}", 0, 1)[:, :D],
                                 start=True, stop=True)
                g = const.tile([P, D], F32, tag=f"gbc{l}")
                nc.vector.tensor_copy(out=g[:, :], in_=g_ps)
                gbc.append(g)
                b_ps = bankA[:, 448: 448 + D]
                nc.tensor.matmul(out=b_ps, lhsT=ones1,
                                 rhs=S(f"lnba{l}", 0, 1)[:, :D],
                                 start=True, stop=True)
                bb = const.tile([P, D], F32, tag=f"bbc{l}")
                nc.vector.tensor_copy(out=bb[:, :], in_=b_ps)
                bbc.append(bb)

        def ln_relu_res(u_ps, m, l, res_ps, xout_ap):
            """xout = relu(LN(u_ps) [*g+b]) + res_ps."""
            stats = tmp.tile([P, 6], F32, tag="stats")
            mv = tmp.tile([P, 2], F32, tag="mv")
            rs = tmp.tile([P, 1], F32, tag="rs")
            nc.vector.bn_stats(out=stats[:m, :], in_=u_ps[:m, :D])
            nc.vector.bn_aggr(out=mv[:m, :], in_=stats[:m, :])
            nc.scalar.activation(out=rs[:m, :], in_=mv[:m, 1:2],
                                 func=mybir.ActivationFunctionType.Sqrt,
                                 bias=eps_t[:m], scale=1.0)
            nc.vector.reciprocal(out=rs[:m, :], in_=rs[:m, :])
            y = tmp.tile([P, D], F32, tag="y")
            nc.vector.tensor_scalar(out=y[:m, :D], in0=u_ps[:m, :D],
                                    scalar1=mv[:m, 0:1], scalar2=rs[:m, 0:1],
                                    op0=mybir.AluOpType.subtract,
                                    op1=mybir.AluOpType.mult)
            if flags["ln_affine"]:
                nc.vector.tensor_mul(out=y[:m, :D], in0=y[:m, :D],
                                     in1=gbc[l][:m, :D])
                nc.vector.tensor_add(out=y[:m, :D], in0=y[:m, :D],
                                     in1=bbc[l][:m, :D])
            nc.vector.scalar_tensor_tensor(out=xout_ap, in0=y[:m, :D],
                                           scalar=0.0, in1=res_ps,
                                           op0=mybir.AluOpType.max,
                                           op1=mybir.AluOpType.add)

        def dense(catT_sb, m, l, w_rhs, u_ps):
            nc.tensor.matmul(out=u_ps[:m, :D], lhsT=catT_sb[: 2 * D, :m],
                             rhs=w_rhs, start=True,
                             stop=not flags["layer_bias"])
            if flags["layer_bias"]:
                nc.tensor.matmul(out=u_ps[:m, :D],
                                 lhsT=ones_row[:, :1].to_broadcast([1, m]),
                                 rhs=S(f"lb{l}", 0, 1)[:, :D],
                                 start=False, stop=True)
            return u_ps

        # ---- PSUM regions
        cat1_ps = bankA[:, 0:P]
        cat2_ps = bankD[:, 0:P]
        cat3_ps = bankE[:, 0:P]
        u_regs = [bankB[:, 0:D], bankB[:, D: 2 * D], bankB[:, 2 * D: 3 * D]]
        h_ps = bankB[:, 192: 192 + P]
        qwb_ps = bankB[:, 320: 320 + D]
        sc_ps = bankB[:, 384: 384 + P]
        x0_ps = bankC[:, 0:D]
        gxp2_ps = bankC[:, D: 2 * D]     # rows 0:Q2 = gx2, Q2:Q2+M2 = xp2
        gxp3_ps = bankC[:, 2 * D: 3 * D]
        tr2_ps = bankC[:, 3 * D: 4 * D]
        tr3_ps = bankC[:, 4 * D: 5 * D]

        # =========== PE queue, data-arrival order ==========================
        # layer-1 chain head (blob only)
        nc.tensor.matmul(out=cat1_ps[:D, :M1], lhsT=S("rel", 0, RU),
                         rhs=S("c1", 0, RU), start=True, stop=True)
        nc.tensor.matmul(out=cat1_ps[D: 2 * D, :M1], lhsT=qv_row,
                         rhs=h01_row, start=True, stop=True)
        nc.tensor.matmul(out=x0_ps[:M1, :D], lhsT=h01_row, rhs=qv_row,
                         start=True, stop=True)
        # structure-only matmuls (blob only): trel, qwb, cat boundary inits
        nc.tensor.matmul(out=tr2_ps[:Q2, :D], lhsT=S("tm2", 0, RU),
                         rhs=S("rel", 0, RU), start=True, stop=True)
        nc.tensor.matmul(out=tr3_ps[:Q3, :D], lhsT=S("tm3", 0, RU),
                         rhs=S("rel", 0, RU), start=True, stop=True)
        nc.tensor.matmul(out=qwb_ps[:1, :D], lhsT=S("qvTb", 0, D),
                         rhs=S("mw1b", 0, D), start=True, stop=True)
        # PSUM->SBUF staging for the per-edge relation rows (DVE may read
        # only one PSUM operand per op); Act engine is idle here.
        trel_sb = []
        for l, (ps, Q) in enumerate(((tr2_ps, Q2), (tr3_ps, Q3))):
            t = const.tile([P, D], F32, tag=f"trel{l}")
            nc.scalar.copy(out=t[:Q, :D], in_=ps[:Q, :D])
            trel_sb.append(t)

        catT1 = tmp.tile([P, P], BF16, tag="catT1")
        nc.vector.tensor_copy(out=catT1[: 2 * D, :M1],
                              in_=cat1_ps[: 2 * D, :M1])
        u1_ps = dense(catT1, M1, 0, w0q[: 2 * D, :D], u_regs[0])
        x1 = const.tile([P, D], BF16, tag="x1")
        ln_relu_res(u1_ps, M1, 0, x0_ps[:M1, :D], x1[:M1, :D])

        qwb = const.tile([1, D], BF16, tag="qwb")
        nc.scalar.copy(out=qwb[:1, :D], in_=qwb_ps[:1, :D])

        def mp_layer(li, x_prev, Mp, Q, Mn, gg_name, s_name, h0_row,
                     w_rhs, cat_ps, gxp_ps, xout_ap):
            XO = ((Q + 31) // 32) * 32
            # one matmul -> [gx ; 0 ; xp] stacked ([G_src |0| G_prev] fused)
            nc.tensor.matmul(out=gxp_ps[: XO + Mn, :D],
                             lhsT=S(gg_name, 0, Mp), rhs=x_prev[:Mp, :D],
                             start=True, stop=True)
            nc.tensor.matmul(out=cat_ps[D: 2 * D, :Mn], lhsT=x_prev[:Mp, :D],
                             rhs=S(gg_name, 0, Mp)[:, XO: XO + Mn],
                             start=True, stop=True)
            msg = const.tile([P, D], BF16, tag=f"msg{li}")
            nc.vector.tensor_mul(out=msg[:Q, :D], in0=gxp_ps[:Q, :D],
                                 in1=trel_sb[li - 2][:Q, :D])
            nc.tensor.matmul(out=cat_ps[:D, :Mn], lhsT=msg[:Q, :D],
                             rhs=S(s_name, 0, Q), start=True, stop=False)
            nc.tensor.matmul(out=cat_ps[:D, :Mn], lhsT=qv_row, rhs=h0_row,
                             start=False, stop=True)
            catT = tmp.tile([P, P], BF16, tag=f"catT{li}")
            nc.vector.tensor_copy(out=catT[: 2 * D, :Mn],
                                  in_=cat_ps[: 2 * D, :Mn])
            u_ps = dense(catT, Mn, li - 1, w_rhs, u_regs[li - 1])
            ln_relu_res(u_ps, Mn, li - 1, gxp_ps[XO: XO + Mn, :D], xout_ap)

        x2 = const.tile([P, D], BF16, tag="x2")
        mp_layer(2, x1, M1, Q2, M2, "gg2", "s2", h02_row,
                 S("w1", 0, 2 * D), cat2_ps, gxp2_ps, x2[:M2, :D])
        mp_layer(3, x2, M2, Q3, KC, "gg3", "s3", h03_row,
                 S("w2", 0, 2 * D), cat3_ps, gxp3_ps, x3[:KC, :D])

        # =========== final MLP:
        #   sc = mw2^T relu(mw1top^T x3^T + qwb^T (x) 1) [+ b]
        x3T = const.tile([32, D], BF16, tag="x3T")
        nc.vector.transpose(out=x3T[:, :], in_=x3[:, :])
        nc.tensor.matmul(out=h_ps[:D, :KC], lhsT=qwb[:1, :D],
                         rhs=ones_row, start=True, stop=False)
        nc.tensor.matmul(out=h_ps[:D, :KC], lhsT=S("mw1h", 0, 32)[:, :D],
                         rhs=x3T[0:32, 0:KC], start=False,
                         stop=False)
        nc.tensor.matmul(out=h_ps[:D, :KC],
                         lhsT=S("mw1h", 0, 32)[:, D: 2 * D],
                         rhs=x3T[0:32, 32: 32 + KC], start=False,
                         stop=not flags["mlp_bias"])
        if flags["mlp_bias"]:
            nc.tensor.matmul(out=h_ps[:D, :KC],
                             lhsT=S("mb1", 0, 1)[:, :D],
                             rhs=ones_row, start=False, stop=True)
        h = tmp.tile([D, P], BF16, tag="h")
        nc.vector.tensor_scalar_max(out=h[:D, :KC], in0=h_ps[:D, :KC],
                                    scalar1=0.0)
        nc.tensor.matmul(out=sc_ps[:1, :KC], lhsT=S("mw2", 0, D),
                         rhs=h[:D, :KC], start=True, stop=True)
        if use_trig:
            if flags["mlp_bias"]:
                nc.vector.tensor_scalar_add(out=scsrc[0:1, :KC],
                                            in0=sc_ps[:1, :KC],
                                            scalar1=float(mlp_b2))
            else:
                nc.vector.tensor_copy(out=scsrc[0:1, :KC],
                                      in_=sc_ps[:1, :KC])
            # the prep's deferred source-read dep resolved against the state
            # at prep EMISSION (the memset), not the copy above: gate the
            # trigger on the copy via a same-engine DRAIN.
            nc.vector.drain().then_inc(csem, 1)
            nc.gpsimd.wait_ge(csem, 1)
            nc.gpsimd.trigger_dma(count=None)
            nc.sync.wait_ge(dma_sem, 16)
        else:
            if flags["mlp_bias"]:
                nc.vector.tensor_scalar_add(out=scsrc[0:1, :KC],
                                            in0=sc_ps[:1, :KC],
                                            scalar1=float(mlp_b2))
            else:
                nc.vector.tensor_copy(out=scsrc[0:1, :KC],
                                      in_=sc_ps[:1, :KC])
            nc.sync.dma_start(out=score[0:1, :KC], in_=scsrc[0:1, :KC])

    # The Tile epilogue also waits the DMASW lane clock, but for a
    # prepare_only prep with a user completion sem nothing ever bumps that
    # lane sem (completion is user-managed above).  Drop the dangling waits.
    updated = set()
    for inst in nc.inst_map.values():
        si = inst.sync_info
        if si is None:
            continue
        for u in si.on_update:
            if u.ant_name:
                updated.add(u.ant_name)
    for inst in nc.inst_map.values():
        si = inst.sync_info
        if si is None:
            continue
        dangling = [w for w in si.on_wait
                    if w.ant_name and w.ant_name.startswith("DMASW")
                    and w.ant_name not in updated]
        if dangling:
            si.on_wait = [w for w in si.on_wait if w not in dangling]

    nc.finalize()
    return nc


# --------------------------------------------------------------------------
# general fallback: block-tiled variant (any frontier size), full rel table
# --------------------------------------------------------------------------

def _core_structs_general(rel, pb, ci, dims, graph):
    src, dst, et = graph
    M1, Q1, M2, Q2, Q3, KC = (dims[k] for k in
                              ("M1", "Q1", "M2", "Q2", "Q3", "KC"))
    h0, r0, e1, V1 = pb["h0"], pb["r0"], pb["e1"], pb["V1"]
    Tc, e2, e3, V2 = ci["Tc"], ci["e2"], ci["e3"], ci["V2"]
    b = ci["b"]
    R = rel.shape[1]
    pos1 = {n: i for i, n in enumerate(V1)}
    pos2 = {n: i for i, n in enumerate(V2)}
    q1, q2, q3, kc = len(e1), len(e2), len(e3), len(Tc)

    rel_pad = np.zeros((RP, D), np.float32)
    rel_pad[:R] = rel[b]
    r0hot = np.zeros((RP, 1), np.float32)
    r0hot[r0, 0] = 1.0
    h0i1 = np.zeros((1, M1), np.float32)
    h0i1[0, pos1[h0]] = 1.0

    S1T = np.zeros((Q1, M1), np.float32)
    Tm1T = np.zeros((RP, Q1), np.float32)
    if q1:
        S1T[np.arange(q1), [pos1[n] for n in dst[e1]]] = 1.0
        Tm1T[et[e1], np.arange(q1)] = 1.0
    G2T = np.zeros((M1, Q2), np.float32)
    S2T = np.zeros((Q2, M2), np.float32)
    Tm2T = np.zeros((RP, Q2), np.float32)
    if q2:
        G2T[[pos1[n] for n in src[e2]], np.arange(q2)] = 1.0
        S2T[np.arange(q2), [pos2[n] for n in dst[e2]]] = 1.0
        Tm2T[et[e2], np.arange(q2)] = 1.0
    G12T = np.zeros((M1, M2), np.float32)
    for n in V2:
        if n in pos1:
            G12T[pos1[n], pos2[n]] = 1.0
    h0i2 = np.zeros((1, M2), np.float32)
    if h0 in pos2:
        h0i2[0, pos2[h0]] = 1.0
    G3T = np.zeros((M2, Q3), np.float32)
    S3T = np.zeros((Q3, KC), np.float32)
    Tm3T = np.zeros((RP, Q3), np.float32)
    if q3:
        G3T[[pos2[n] for n in src[e3]], np.arange(q3)] = 1.0
        S3T[:q3, :kc] = (dst[e3][:, None] == Tc[None, :]).astype(np.float32)
        Tm3T[et[e3], np.arange(q3)] = 1.0
    G23T = np.zeros((M2, KC), np.float32)
    G23T[[pos2[n] for n in Tc], np.arange(kc)] = 1.0
    h0i3 = np.zeros((1, KC), np.float32)
    h0i3[0, :kc] = (Tc == h0).astype(np.float32)

    return dict(rel=rel_pad, r0hot=r0hot, S1T=S1T, Tm1T=Tm1T, h0ind1=h0i1,
                G2T=G2T, S2T=S2T, Tm2T=Tm2T, G12T=G12T, h0ind2=h0i2,
                G3T=G3T, S3T=S3T, Tm3T=Tm3T, G23T=G23T, h0ind3=h0i3)


def _core_in_map_general(inputs, rel, pb, ci, dims, graph):
    im = _core_structs_general(rel, pb, ci, dims, graph)
    im.update(
        layer_w=np.ascontiguousarray(np.asarray(inputs["layer_w"], np.float32)),
        layer_b=np.ascontiguousarray(np.asarray(inputs["layer_b"], np.float32)),
        ln_g=np.ascontiguousarray(np.asarray(inputs["ln_g"], np.float32)),
        ln_b=np.ascontiguousarray(np.asarray(inputs["ln_b"], np.float32)),
        mlp_w1=np.ascontiguousarray(np.asarray(inputs["mlp_w1"], np.float32)),
        mlp_b1=np.asarray(inputs["mlp_b1"], np.float32).reshape(D, 1).copy(),
        mlp_w2=np.ascontiguousarray(np.asarray(inputs["mlp_w2"], np.float32)),
        mlp_b2=np.asarray(inputs["mlp_b2"], np.float32).reshape(1, 1).copy(),
    )
    return im


def _build_nc_general(dims):
    M1, Q1, M2, Q2, Q3, KC = (dims[k] for k in
                              ("M1", "Q1", "M2", "Q2", "Q3", "KC"))
    nc = bacc.Bacc()

    def din(name, shape):
        return nc.declare_dram_parameter(name, list(shape), F32, isOutput=False)

    rel = din("rel", (RP, D))
    r0hot = din("r0hot", (RP, 1))
    lw = din("layer_w", (L, 2 * D, D))
    lb = din("layer_b", (L, D))
    lng = din("ln_g", (L, D))
    lnb = din("ln_b", (L, D))
    w1 = din("mlp_w1", (2 * D, D))
    b1 = din("mlp_b1", (D, 1))
    w2 = din("mlp_w2", (D, 1))
    b2 = din("mlp_b2", (1, 1))
    s1t = din("S1T", (Q1, M1))
    tm1 = din("Tm1T", (RP, Q1))
    h01 = din("h0ind1", (1, M1))
    g2t = din("G2T", (M1, Q2))
    s2t = din("S2T", (Q2, M2))
    tm2 = din("Tm2T", (RP, Q2))
    g12 = din("G12T", (M1, M2))
    h02 = din("h0ind2", (1, M2))
    g3t = din("G3T", (M2, Q3))
    s3t = din("S3T", (Q3, KC))
    tm3 = din("Tm3T", (RP, Q3))
    g23 = din("G23T", (M2, KC))
    h03 = din("h0ind3", (1, KC))
    score = nc.declare_dram_parameter("score", [1, KC], F32, isOutput=True)

    with ExitStack() as ctx:
        tc = ctx.enter_context(tile.TileContext(nc))
        const = ctx.enter_context(tc.tile_pool(name="const", bufs=1))
        tmp = ctx.enter_context(tc.tile_pool(name="tmp", bufs=2))
        pps = ctx.enter_context(tc.tile_pool(name="ps", bufs=2, space="PSUM"))

        ident = const.tile([P, P], F32, tag="ident")
        make_identity(nc, ident[:])
        ones_row = const.tile([1, P], F32, tag="ones_row")
        nc.vector.memset(ones_row[:], 1.0)
        eps_t = const.tile([P, 1], F32, tag="eps")
        nc.vector.memset(eps_t[:], EPS)

        def load(dram, rows, cols, tag):
            out = []
            for i, (o, sz) in enumerate(_blk(rows)):
                t = const.tile([P, cols], F32, tag=f"{tag}{i}")
                nc.sync.dma_start(out=t[:sz, :cols], in_=dram[o: o + sz, 0:cols])
                out.append((t, sz))
            return out

        rel_b = load(rel, RP, D, "rel")
        r0h_b = load(r0hot, RP, 1, "r0h")
        tm1_b = load(tm1, RP, Q1, "tm1")
        tm2_b = load(tm2, RP, Q2, "tm2")
        tm3_b = load(tm3, RP, Q3, "tm3")
        s1t_b = load(s1t, Q1, M1, "s1t")
        s2t_b = load(s2t, Q2, M2, "s2t")
        s3t_b = load(s3t, Q3, KC, "s3t")
        g2t_b = load(g2t, M1, Q2, "g2t")
        g12_b = load(g12, M1, M2, "g12")
        g3t_b = load(g3t, M2, Q3, "g3t")
        g23_b = load(g23, M2, KC, "g23")
        h01_sb = load(h01, 1, M1, "h01")[0][0]
        h02_sb = load(h02, 1, M2, "h02")[0][0]
        h03_sb = load(h03, 1, KC, "h03")[0][0]

        w_sb = [load(lw[l], 2 * D, D, f"w{l}")[0][0] for l in range(L)]
        lb_sb = [load(lb[l: l + 1], 1, D, f"lb{l}")[0][0] for l in range(L)]
        w1_sb = load(w1, 2 * D, D, "w1")[0][0]
        b1_sb = load(b1, D, 1, "b1")[0][0]
        w2_sb = load(w2, D, 1, "w2")[0][0]
        b2_sb = load(b2, 1, 1, "b2")[0][0]

        gbc, bbc = [], []
        for l in range(L):
            g = const.tile([P, D], F32, tag=f"gbc{l}")
            nc.sync.dma_start(out=g[:, :D], in_=lng[l].partition_broadcast(P))
            gbc.append(g)
            bb = const.tile([P, D], F32, tag=f"bbc{l}")
            nc.sync.dma_start(out=bb[:, :D], in_=lnb[l].partition_broadcast(P))
            bbc.append(bb)

        qv_ps = pps.tile([1, D], F32, tag="ps_c")
        for i, ((rt, rs), (ht, _)) in enumerate(zip(rel_b, r0h_b)):
            nc.tensor.matmul(out=qv_ps[:1, :D], lhsT=ht[:rs, :1], rhs=rt[:rs, :D],
                             start=(i == 0), stop=(i == len(rel_b) - 1))
        qv = const.tile([1, D], F32, tag="qv")
        nc.vector.tensor_copy(out=qv[:1, :D], in_=qv_ps[:1, :D])

        qvT_ps = pps.tile([D, 1], F32, tag="ps_c")
        for i, ((rt, rs), (ht, _)) in enumerate(zip(rel_b, r0h_b)):
            nc.tensor.matmul(out=qvT_ps[:D, :1], lhsT=rt[:rs, :D], rhs=ht[:rs, :1],
                             start=(i == 0), stop=(i == len(rel_b) - 1))
        qvT = const.tile([D, 1], F32, tag="qvT")
        nc.vector.tensor_copy(out=qvT[:D, :1], in_=qvT_ps[:D, :1])

        qbc_ps = pps.tile([P, D], F32, tag="ps_a")
        nc.tensor.matmul(out=qbc_ps[:P, :D], lhsT=ones_row[:1, :P], rhs=qv[:1, :D],
                         start=True, stop=True)
        qbc = const.tile([P, D], F32, tag="qbc")
        nc.vector.tensor_copy(out=qbc[:, :D], in_=qbc_ps[:, :D])

        def ln_relu_res(u, ms, l, xprev, xout):
            stats = tmp.tile([P, 6], F32, tag="stats")
            mv = tmp.tile([P, 2], F32, tag="mv")
            nc.vector.bn_stats(out=stats[:ms, :], in_=u[:ms, :D])
            nc.vector.bn_aggr(out=mv[:ms, :], in_=stats[:ms, :])
            mean = mv[:ms, 0:1]
            var = mv[:ms, 1:2]
            nc.scalar.activation(out=var, in_=var,
                                 func=mybir.ActivationFunctionType.Sqrt,
                                 bias=eps_t[:ms], scale=1.0)
            nc.vector.reciprocal(out=var, in_=var)
            nc.vector.tensor_scalar(out=u[:ms, :D], in0=u[:ms, :D],
                                    scalar1=mean, scalar2=var,
                                    op0=mybir.AluOpType.subtract,
                                    op1=mybir.AluOpType.mult)
            nc.vector.tensor_mul(out=u[:ms, :D], in0=u[:ms, :D], in1=gbc[l][:ms, :D])
            nc.vector.tensor_add(out=u[:ms, :D], in0=u[:ms, :D], in1=bbc[l][:ms, :D])
            nc.vector.scalar_tensor_tensor(out=xout[:ms, :D], in0=u[:ms, :D],
                                           scalar=0.0, in1=xprev[:ms, :D],
                                           op0=mybir.AluOpType.max,
                                           op1=mybir.AluOpType.add)

        def dense_update(xcat, ms, l, xprev, xout):
            xT_ps = pps.tile([P, P], F32, tag="ps_b")
            nc.tensor.transpose(out=xT_ps[: 2 * D, :ms], in_=xcat[:ms, : 2 * D],
                                identity=ident[:ms, :ms])
            xT = tmp.tile([P, P], F32, tag="xT")
            nc.vector.tensor_copy(out=xT[: 2 * D, :ms], in_=xT_ps[: 2 * D, :ms])
            upd_ps = pps.tile([P, D], F32, tag="ps_a")
            nc.tensor.matmul(out=upd_ps[:ms, :D], lhsT=xT[: 2 * D, :ms],
                             rhs=w_sb[l][: 2 * D, :D], start=True, stop=False)
            nc.tensor.matmul(out=upd_ps[:ms, :D], lhsT=ones_row[:1, :ms],
                             rhs=lb_sb[l][:1, :D], start=False, stop=True)
            u = tmp.tile([P, D], F32, tag="u")
            nc.vector.tensor_copy(out=u[:ms, :D], in_=upd_ps[:ms, :D])
            ln_relu_res(u, ms, l, xprev, xout)

        def msgs(tm_b, g_b, x_blocks, Q, tag):
            out = []
            for j, (qo, qs) in enumerate(_blk(Q)):
                tr_ps = pps.tile([P, D], F32, tag="ps_a")
                for i, (rt, rs) in enumerate(rel_b):
                    nc.tensor.matmul(out=tr_ps[:qs, :D],
                                     lhsT=tm_b[i][0][:rs, qo: qo + qs],
                                     rhs=rt[:rs, :D],
                                     start=(i == 0), stop=(i == len(rel_b) - 1))
                m = const.tile([P, D], F32, tag=f"{tag}_{j}")
                if x_blocks is None:
                    nc.vector.tensor_mul(out=m[:qs, :D], in0=tr_ps[:qs, :D],
                                         in1=qbc[:qs, :D])
                else:
                    gx_ps = pps.tile([P, D], F32, tag="ps_b")
                    for i, (xt, ms_) in enumerate(x_blocks):
                        nc.tensor.matmul(out=gx_ps[:qs, :D],
                                         lhsT=g_b[i][0][:ms_, qo: qo + qs],
                                         rhs=xt[:ms_, :D],
                                         start=(i == 0),
                                         stop=(i == len(x_blocks) - 1))
                    gx = tmp.tile([P, D], F32, tag="gx")
                    nc.vector.tensor_copy(out=gx[:qs, :D], in_=gx_ps[:qs, :D])
                    nc.vector.tensor_mul(out=m[:qs, :D], in0=tr_ps[:qs, :D],
                                         in1=gx[:qs, :D])
                out.append((m, qs))
            return out

        def aggregate(s_b, msg_blocks, h0_sb, mo, ms):
            agg_ps = pps.tile([P, D], F32, tag="ps_a")
            nc.tensor.matmul(out=agg_ps[:ms, :D], lhsT=h0_sb[:1, mo: mo + ms],
                             rhs=qv[:1, :D], start=True, stop=False)
            for j, (mt, qs) in enumerate(msg_blocks):
                nc.tensor.matmul(out=agg_ps[:ms, :D],
                                 lhsT=s_b[j][0][:qs, mo: mo + ms], rhs=mt[:qs, :D],
                                 start=False, stop=(j == len(msg_blocks) - 1))
            return agg_ps

        def gather_nodes(g_b, x_blocks, mo, ms, tag):
            ps = pps.tile([P, D], F32, tag="ps_b")
            for i, (xt, ms_) in enumerate(x_blocks):
                nc.tensor.matmul(out=ps[:ms, :D], lhsT=g_b[i][0][:ms_, mo: mo + ms],
                                 rhs=xt[:ms_, :D],
                                 start=(i == 0), stop=(i == len(x_blocks) - 1))
            t = const.tile([P, D], F32, tag=tag)
            nc.vector.tensor_copy(out=t[:ms, :D], in_=ps[:ms, :D])
            return t

        # layer 1
        msg1 = msgs(tm1_b, None, None, Q1, "msg1")
        x1 = []
        for mi, (mo, ms) in enumerate(_blk(M1)):
            agg_ps = aggregate(s1t_b, msg1, h01_sb, mo, ms)
            x0_ps = pps.tile([P, D], F32, tag="ps_b")
            nc.tensor.matmul(out=x0_ps[:ms, :D], lhsT=h01_sb[:1, mo: mo + ms],
                             rhs=qv[:1, :D], start=True, stop=True)
            x0 = const.tile([P, D], F32, tag=f"x0_{mi}")
            nc.vector.tensor_copy(out=x0[:ms, :D], in_=x0_ps[:ms, :D])
            xcat = tmp.tile([P, 2 * D], F32, tag="xcat")
            nc.vector.tensor_copy(out=xcat[:ms, :D], in_=agg_ps[:ms, :D])
            nc.vector.tensor_copy(out=xcat[:ms, D: 2 * D], in_=x0[:ms, :D])
            xo = const.tile([P, D], F32, tag=f"x1_{mi}")
            dense_update(xcat, ms, 0, x0, xo)
            x1.append((xo, ms))

        # layer 2
        msg2 = msgs(tm2_b, g2t_b, x1, Q2, "msg2")
        x2 = []
        for mi, (mo, ms) in enumerate(_blk(M2)):
            agg_ps = aggregate(s2t_b, msg2, h02_sb, mo, ms)
            xp = gather_nodes(g12_b, x1, mo, ms, f"x1v2_{mi}")
            xcat = tmp.tile([P, 2 * D], F32, tag="xcat")
            nc.vector.tensor_copy(out=xcat[:ms, :D], in_=agg_ps[:ms, :D])
            nc.vector.tensor_copy(out=xcat[:ms, D: 2 * D], in_=xp[:ms, :D])
            xo = const.tile([P, D], F32, tag=f"x2_{mi}")
            dense_update(xcat, ms, 1, xp, xo)
            x2.append((xo, ms))

        # layer 3 (target slots)
        msg3 = msgs(tm3_b, g3t_b, x2, Q3, "msg3")
        x3 = []
        for mi, (mo, ms) in enumerate(_blk(KC)):
            agg_ps = aggregate(s3t_b, msg3, h03_sb, mo, ms)
            xp = gather_nodes(g23_b, x2, mo, ms, f"x2v3_{mi}")
            xcat = tmp.tile([P, 2 * D], F32, tag="xcat")
            nc.vector.tensor_copy(out=xcat[:ms, :D], in_=agg_ps[:ms, :D])
            nc.vector.tensor_copy(out=xcat[:ms, D: 2 * D], in_=xp[:ms, :D])
            xo = const.tile([P, D], F32, tag=f"x3_{mi}")
            dense_update(xcat, ms, 2, xp, xo)
            x3.append((xo, ms))

        # final MLP
        for (x3t, ms), (mo, _) in zip(x3, _blk(KC)):
            x3T_ps = pps.tile([P, P], F32, tag="ps_b")
            nc.tensor.transpose(out=x3T_ps[:D, :ms], in_=x3t[:ms, :D],
                                identity=ident[:ms, :ms])
            featT = tmp.tile([P, P], F32, tag="featT")
            nc.vector.tensor_copy(out=featT[:D, :ms], in_=x3T_ps[:D, :ms])
            nc.vector.tensor_copy(out=featT[D: 2 * D, :ms],
                                  in_=qvT[:D, :1].to_broadcast([D, ms]))
            h_ps = pps.tile([D, P], F32, tag="ps_a")
            nc.tensor.matmul(out=h_ps[:D, :ms], lhsT=w1_sb[: 2 * D, :D],
                             rhs=featT[: 2 * D, :ms], start=True, stop=True)
            h = tmp.tile([D, P], F32, tag="h")
            nc.vector.tensor_scalar(out=h[:D, :ms], in0=h_ps[:D, :ms],
                                    scalar1=b1_sb[:D, :1], scalar2=None,
                                    op0=mybir.AluOpType.add)
            nc.vector.tensor_scalar_max(out=h[:D, :ms], in0=h[:D, :ms],
                                        scalar1=0.0)
            sc_ps = pps.tile([1, P], F32, tag="ps_c")
            nc.tensor.matmul(out=sc_ps[:1, :ms], lhsT=w2_sb[:D, :1],
                             rhs=h[:D, :ms], start=True, stop=True)
            sc = tmp.tile([1, P], F32, tag="sc")
            nc.vector.tensor_scalar(out=sc[:1, :ms], in0=sc_ps[:1, :ms],
                                    scalar1=b2_sb[:1, :1], scalar2=None,
                                    op0=mybir.AluOpType.add)
            nc.sync.dma_start(out=score[0:1, mo: mo + ms], in_=sc[:1, :ms])

    nc.finalize()
    return nc


# --------------------------------------------------------------------------
# numpy fallback (zero-bias structural assumption violated)
# --------------------------------------------------------------------------

def _dense_numpy(inputs):
    rel = np.asarray(inputs["relation_representations"], np.float32)
    lw = np.asarray(inputs["layer_w"], np.float32)
    lbv = np.asarray(inputs["layer_b"], np.float32)
    lng = np.asarray(inputs["ln_g"], np.float32)
    lnb = np.asarray(inputs["ln_b"], np.float32)
    batch = np.asarray(inputs["batch"])
    ei = np.asarray(inputs["edge_index"])
    et = np.asarray(inputs["edge_type"])
    N = int(inputs["num_nodes"])
    B = rel.shape[0]
    h0 = batch[:, 0, 0].astype(np.int64)
    r0 = batch[:, 0, 2].astype(np.int64)
    t = batch[:, :, 1].astype(np.int64)
    query = rel[np.arange(B), r0]
    boundary = np.zeros((B, N, rel.shape[2]), np.float32)
    boundary[np.arange(B), h0] += query
    src, dst = ei[0], ei[1]
    x = boundary.copy()
    for l in range(lw.shape[0]):
        msg = x[:, src] * rel[:, et]
        agg = np.zeros_like(x)
        np.add.at(agg, (slice(None), dst), msg)
        agg += boundary
        u = np.concatenate([agg, x], -1) @ lw[l] + lbv[l]
        mu = u.mean(-1, keepdims=True)
        var = ((u - mu) ** 2).mean(-1, keepdims=True)
        u = (u - mu) / np.sqrt(var + EPS) * lng[l] + lnb[l]
        x = np.maximum(u, 0) + x
    feat_t = np.take_along_axis(
        np.concatenate([x, np.broadcast_to(query[:, None, :], x.shape)], -1),
        t[..., None], axis=1)
    w1 = np.asarray(inputs["mlp_w1"], np.float32)
    b1 = np.asarray(inputs["mlp_b1"], np.float32)
    w2 = np.asarray(inputs["mlp_w2"], np.float32)
    b2 = np.asarray(inputs["mlp_b2"], np.float32)
    return ((np.maximum(feat_t @ w1 + b1, 0) @ w2 + b2)[..., 0]).astype(np.float32)


# --------------------------------------------------------------------------
# public entry
# --------------------------------------------------------------------------

def kernel(**inputs) -> np.ndarray:
    rel = np.asarray(inputs["relation_representations"], np.float32)
    batch = np.asarray(inputs["batch"])
    B, K = batch.shape[0], batch.shape[1]
    R = rel.shape[1]

    # zero-row invariance needs layer_b == ln_b == 0 (true per the input spec)
    if (not (np.all(np.asarray(inputs["layer_b"]) == 0)
             and np.all(np.asarray(inputs["ln_b"]) == 0))
            or N_CORES % B or rel.shape[2] != D or R >= RP):
        return _dense_numpy(inputs)

    per_batch, cores, dims, graph = _prep_host(
        rel, batch, np.asarray(inputs["edge_index"]),
        np.asarray(inputs["edge_type"]))
    flags = _flags(inputs)
    flags["mlp_b2_val"] = float(np.asarray(inputs["mlp_b2"]).reshape(-1)[0])

    fast = (max(dims["M1"], dims["M2"], dims["Q2"], dims["Q3"]) <= P
            and dims["RU"] <= P and dims["KC"] <= 32)
    if not fast and max(dims.values()) > 4096:
        return _dense_numpy(inputs)  # pathological hub graph: stay correct

    if fast:
        in_maps, lay = _prep_blobs(
            inputs, rel, per_batch, cores, dims, graph, flags)
        nc = _build_nc_fast(dims, lay, flags)
    else:
        in_maps = [_core_in_map_general(inputs, rel, per_batch[ci["b"]], ci,
                                        dims, graph) for ci in cores]
        nc = _build_nc_general(dims)

    res = run_bass_kernel_spmd(nc, in_maps, list(range(N_CORES)))
    out = np.zeros((B, K), np.float32)
    for c, ci in enumerate(cores):
        out[ci["b"], ci["chunk"]] = res.results[c]["score"][0, : len(ci["Tc"])]
    return out
